# revision 40
# baseline (speedup 1.0000x reference)
"""Bi-Real Net binary conv2d (3x3, pad 1, stride 1) for Trainium2, 8 NeuronCores.

Math (forward values of the reference):
    xb = sign(x)                      in {-1, 0, +1}
    scale[o] = mean_{i,kh,kw} |w[o,i,kh,kw]|
    wb = scale[o] * sign(w)
    y = conv2d_NCHW(xb, wb, pad=1)

Kernel strategy:
    - Data-parallel over batch: 32 images -> 4 per core on 8 cores.
    - Per image: DMA [128, 112*112] f32 -> SBUF, ACT Sign -> zero-padded
      bf16 buffer [128, 114, 114].
    - Conv as 9 accumulated matmuls per 4-output-row chunk:
      psum[o, 4x112] += signW_tap[i, o].T @ xpad[i, rows+kh, kw:kw+112].
      Products are +-1 in bf16 (exact); PSUM accumulates exact integers.
    - PSUM evacuation on DVE multiplies by per-channel scale[o] (fp32).
    - Outputs staged in SBUF (16 rows) and DMA'd out in ~0.9 MB chunks.
"""

import sys

sys.path.insert(0, "/opt/trn_rl_repo")

import numpy as np

import concourse.bacc as bacc
import concourse.bass as bass
import concourse.mybir as mybir
import concourse.tile as tile
from concourse.bass_utils import run_bass_kernel_spmd
from concourse.masks import make_identity

N_CORES = 8
B, C, H, W = 32, 128, 112, 112
BL = B // N_CORES  # images per core
HP = H + 2  # padded height/width (114)
TAPS = [(kh, kw) for kh in range(3) for kw in range(3)]

F32 = mybir.dt.float32
BF16 = mybir.dt.bfloat16

N_ROWCHUNK = 4  # output rows per PSUM accumulation group (<= one 2KB bank)
N_STAGEROWS = 28  # output rows per SBUF->DRAM store (must divide 112)
N_LOADROWS = 28  # input rows per DRAM->SBUF load
N_SIGNROWS = 14  # input rows per ACT Sign instruction


RP = 128  # fp8 padded-row pitch; 128 makes the DoubleRow mid-dim step %16==0

VARIANT = "v7"  # "bf16" | "fp8dr" | "fp8dr5" | "v2" | "v3" | "v4" | "v5"


def build_nc_v4(variant="v4"):
    """v3 + row-interleaved P1 plane and 4-level matmul rhs APs.

    v3's plane-pair DoubleRow matmul streamed ~1.75x slower than the
    vertical pairs (pair elements 14.7KB apart in SBUF). Interleaving
    P0/P1 rows ([C, nrows, 2, pitch]) makes every pair step 128 or 256
    bytes. The rhs then needs [pair][row][col] = 4-level APs, which also
    lets all matmuls stream only the 448 useful columns (no garbage
    columns), and PSUM chunks become contiguous 448-col banks.

    v5: startup/tail polish — image-0 P1 pieces on DVE (ACT was the
    warmup serializer), weight abs/scale on DVE (shortens the ACT chain
    to the first sign), last image staged in 14-row pieces (shorter
    drain tail).
    """
    v5 = variant in ("v5", "v6", "v7")
    v6 = variant in ("v6", "v7")
    v7 = variant == "v7"
    FP8 = mybir.dt.float8e4
    U16 = mybir.dt.uint16
    pitch = RP
    nrows = HP + 1  # 115
    rstride = 2 * pitch  # consecutive-row stride in the interleaved layout
    stage_rows = N_STAGEROWS

    nc = bacc.Bacc(
        "TRN2", target_bir_lowering=False, debug=False, num_devices=N_CORES
    )
    x = nc.declare_dram_parameter("x", [BL, C, H, W], F32, isOutput=False)
    w = nc.declare_dram_parameter("weight", [C, C, 3, 3], F32, isOutput=False)
    y = nc.declare_dram_parameter("y", [BL, C, H, W], BF16, isOutput=True)

    with tile.TileContext(nc) as tc:
        with (
            tc.tile_pool(name="consts", bufs=1) as consts,
            tc.tile_pool(name="psum", bufs=1, space="PSUM") as psum_pool,
            tc.tile_pool(name="wprep", bufs=1) as wp,
            tc.tile_pool(name="raw", bufs=1) as raw_pool,
            tc.tile_pool(name="xpad", bufs=1) as xpad_pool,
            tc.tile_pool(name="stage", bufs=3) as stage_pool,
        ):
            wf = wp.tile([C, C, 3, 3], F32)
            nc.sync.dma_start(wf[:, :, :, :], w[:, :, :, :])
            # tiny first loads: the weight DMA shares the 16 SDMA engines
            # with whatever x loads are in flight — keep that company small
            # so weights (the warmup critical path) land early
            load_plan0 = [7, 7, 14, 28, 28, 28] if v7 else [14, 14, 28, 28, 28]
            raws0 = []
            r0 = 0
            for rows in load_plan0:
                raw = raw_pool.tile([C, N_LOADROWS, W], F32, tag="raw", bufs=4)
                nc.sync.dma_start(raw[:, :rows, :], x[0][:, r0 : r0 + rows, :])
                raws0.append((r0, rows, raw))
                r0 += rows

            wdr = consts.tile([C, 3, 2, C], FP8)
            wp2 = consts.tile([C, 2, C], FP8)
            if v7:
                # tap (2,1) paired with a zero weight row: the 5th pass is a
                # DoubleRow like the others (uniform DR chunks issue cleanly)
                w21 = consts.tile([C, 2, C], FP8)
                nc.vector.memset(w21[:, 1, :], 0.0)
            else:
                w21 = consts.tile([C, C], FP8)
            scale = consts.tile([C, 1], F32)
            identity = consts.tile([C, C], BF16)
            make_identity(nc, identity)
            wabs = wp.tile([C, C, 3, 3], F32)
            ssum = wp.tile([C, 1], F32)
            # wsign split by kh so the first transposes start ~0.8us sooner;
            # abs/scale deferred (only needed by the first evacuation) so
            # ACT reaches image-0 signs earlier
            wsign = wp.tile([C, C, 3, 3], BF16)
            if v6 and not v7:
                for kh in range(3):
                    nc.scalar.sign(wsign[:, :, kh, :], wf[:, :, kh, :])
            else:
                # one instruction beats three kh slices (3x864ns vs 1.6us)
                nc.scalar.sign(wsign[:, :, :, :], wf[:, :, :, :])

            def emit_weight_scale():
                nc.scalar.activation(
                    wabs[:, :, :, :],
                    wf[:, :, :, :],
                    mybir.ActivationFunctionType.Abs,
                    accum_out=ssum[:, :],
                )
                nc.scalar.mul(scale[:, :], ssum[:, :], 1.0 / (C * 9))

            if not v6:
                emit_weight_scale()
            for t, (kh, kw) in enumerate(TAPS):
                pst = psum_pool.tile([C, C], BF16, tag="pst", bufs=2)
                nc.tensor.transpose(pst[:, :], wsign[:, :, kh, kw], identity[:, :])
                if kh < 2:
                    dst = wdr[:, kw, kh, :]
                elif kw == 0:
                    dst = wp2[:, 0, :]
                elif kw == 2:
                    dst = wp2[:, 1, :]
                elif v7:
                    dst = w21[:, 0, :]
                else:
                    dst = w21[:, :]
                nc.vector.tensor_copy(dst, pst[:, :])

            # interleaved planes: [C, row, {P0,P1}, pitch]
            # P0[r, 1+c] = sign(x[r-1, c]); P1[r, c] = P0[r, c+2]
            xpads = []
            for k in range(2):
                xp = xpad_pool.tile(
                    [C, nrows, 2, pitch], FP8, tag=f"xpad{k}", name=f"xpad{k}"
                )
                xpads.append(xp)
                nc.gpsimd.memset(xp[:, 0, 0, :], 0.0)
                nc.gpsimd.memset(xp[:, HP - 1 :, 0, :], 0.0)
                nc.gpsimd.memset(xp[:, :, 0, W + 1 : pitch], 0.0)
                nc.gpsimd.memset(xp[:, :, 0, 0], 0.0)
                nc.gpsimd.memset(xp[:, 0:2, 1, :], 0.0)
                nc.gpsimd.memset(xp[:, HP - 1 :, 1, :], 0.0)
                nc.gpsimd.memset(xp[:, :, 1, W - 1 : pitch], 0.0)

            # piece 0 pre-signable early by ACT; early pieces on DVE (short
            # queue coupling); late pieces on the lazy Pool engine
            P1_ENG = ["act", "dve", "dve", "dve", "pool", "pool", "pool", "dve"]
            # image 0: ACT is the warmup serializer (wsign + P0 signs), so
            # its P1 piece goes to DVE as well
            P1_ENG0 = ["dve", "dve", "dve", "dve", "pool", "pool", "pool", "dve"]

            for n in range(BL):
                xim = x[n]
                yim = y[n]
                xpad = xpads[n % 2]
                xflat = xpad.rearrange("p r a c -> p (r a c)")
                if n == 0:
                    loads = raws0
                else:
                    loads = []
                    r0 = 0
                    for rows in [N_LOADROWS] * (H // N_LOADROWS):
                        raw = raw_pool.tile(
                            [C, N_LOADROWS, W], F32, tag="raw", bufs=4
                        )
                        nc.sync.dma_start(
                            raw[:, :rows, :], xim[:, r0 : r0 + rows, :]
                        )
                        loads.append((r0, rows, raw))
                        r0 += rows
                for r0, rows, raw in loads:
                    for a in range(0, rows, N_SIGNROWS):
                        rn = min(N_SIGNROWS, rows - a)
                        rr = r0 + a + 1
                        piece = (r0 + a) // N_SIGNROWS
                        nc.scalar.sign(
                            xpad[:, rr : rr + rn, 0, 1 : 1 + W],
                            raw[:, a : a + rn, :],
                        )
                        if v6 and n == 0 and piece == 1 and a == 0:
                            # scale is first needed once 6 PSUM banks fill
                            # (~6 chunks in); emitting here keeps both the
                            # piece-1 sign and the first evacuation unblocked
                            emit_weight_scale()
                        eng = (P1_ENG0 if (v5 and n == 0) else P1_ENG)[piece]
                        if eng == "act":
                            nc.scalar.sign(
                                xpad[:, rr : rr + rn, 1, 0 : W - 1],
                                raw[:, a : a + rn, 1:W],
                            )
                        else:
                            # rn rows of u16: dst = P1 rows, src = P0 rows
                            # shifted +2 fp8 elements (1 u16 element)
                            db = (rr * 2 + 1) * pitch // 2  # u16 offset
                            sb = rr * 2 * pitch // 2 + 1
                            base = xflat[:, 0:2].bitcast(U16)
                            dst = bass.AP(
                                tensor=base.tensor,
                                offset=base.offset + db,
                                ap=[base.ap[0], [pitch, rn], [1, pitch // 2]],
                            )
                            src = bass.AP(
                                tensor=base.tensor,
                                offset=base.offset + sb,
                                ap=[base.ap[0], [pitch, rn], [1, pitch // 2]],
                            )
                            if eng == "pool":
                                nc.gpsimd.tensor_copy(dst, src)
                            else:
                                nc.vector.tensor_copy(dst, src)

                for s0 in range(0, H, stage_rows):
                    stage = stage_pool.tile([C, stage_rows, W], BF16, tag="stage")
                    for j in range(0, stage_rows, N_ROWCHUNK):
                        h0 = s0 + j
                        NF = N_ROWCHUNK * W  # 448
                        ps = psum_pool.tile([C, NF], F32, tag="ps", bufs=6)

                        def mm_vert(kw, start, stop):
                            base = xpad[:, h0, 0, kw]
                            rhs = bass.AP(
                                tensor=base.tensor,
                                offset=base.offset,
                                ap=[
                                    base.ap[0],
                                    [rstride, 2],
                                    [rstride, N_ROWCHUNK],
                                    [1, W],
                                ],
                            )
                            nc.tensor.matmul(
                                ps[:, :],
                                wdr[:, kw, :, :],
                                rhs,
                                start=start,
                                stop=stop,
                                perf_mode=mybir.MatmulPerfMode.DoubleRow,
                            )

                        def mm_plane(start, stop):
                            # taps (2,0)+(2,2): pair step 128 (adjacent planes)
                            base = xpad[:, h0 + 2, 0, 0]
                            rhs = bass.AP(
                                tensor=base.tensor,
                                offset=base.offset,
                                ap=[
                                    base.ap[0],
                                    [pitch, 2],
                                    [rstride, N_ROWCHUNK],
                                    [1, W],
                                ],
                            )
                            nc.tensor.matmul(
                                ps[:, :],
                                wp2[:, :, :],
                                rhs,
                                start=start,
                                stop=stop,
                                perf_mode=mybir.MatmulPerfMode.DoubleRow,
                            )

                        def mm_single(start, stop):
                            # tap (2,1)
                            base = xpad[:, h0 + 2, 0, 1]
                            if v7:
                                rhs = bass.AP(
                                    tensor=base.tensor,
                                    offset=base.offset,
                                    ap=[
                                        base.ap[0],
                                        [rstride, 2],
                                        [rstride, N_ROWCHUNK],
                                        [1, W],
                                    ],
                                )
                                nc.tensor.matmul(
                                    ps[:, :],
                                    w21[:, :, :],
                                    rhs,
                                    start=start,
                                    stop=stop,
                                    perf_mode=mybir.MatmulPerfMode.DoubleRow,
                                )
                            else:
                                rhs = bass.AP(
                                    tensor=base.tensor,
                                    offset=base.offset,
                                    ap=[base.ap[0], [rstride, N_ROWCHUNK], [1, W]],
                                )
                                nc.tensor.matmul(
                                    ps[:, :], w21[:, :], rhs, start=start, stop=stop
                                )

                        mm_vert(0, True, False)
                        mm_vert(1, False, False)
                        mm_vert(2, False, False)
                        mm_plane(False, False)
                        mm_single(False, True)
                        nc.vector.tensor_scalar_mul(
                            stage[:, j : j + N_ROWCHUNK, :], ps[:, :], scale[:, :]
                        )
                    if v7 and n == BL - 1:
                        # last image's stores on the (idle by now) ACT HWDGE
                        # ring: the SWDGE gpsimd drain was 3.6us of pure tail
                        if s0 == H - stage_rows:
                            for j in range(0, stage_rows, stage_rows // 2):
                                nc.scalar.dma_start(
                                    yim[:, s0 + j : s0 + j + stage_rows // 2, :],
                                    stage[:, j : j + stage_rows // 2, :],
                                )
                        else:
                            nc.scalar.dma_start(
                                yim[:, s0 : s0 + stage_rows, :], stage[:, :, :]
                            )
                    elif n == BL - 1 and s0 == H - stage_rows:
                        if v5:
                            # drain the tail in 4-row pieces so the last
                            # transfer is tiny
                            for j in range(0, stage_rows, N_ROWCHUNK):
                                nc.gpsimd.dma_start(
                                    yim[:, s0 + j : s0 + j + N_ROWCHUNK, :],
                                    stage[:, j : j + N_ROWCHUNK, :],
                                )
                        else:
                            hs = stage_rows // 2
                            nc.gpsimd.dma_start(
                                yim[:, s0 : s0 + hs, :], stage[:, :hs, :]
                            )
                            nc.gpsimd.dma_start(
                                yim[:, s0 + hs : s0 + stage_rows, :],
                                stage[:, hs:, :],
                            )
                    else:
                        nc.gpsimd.dma_start(
                            yim[:, s0 : s0 + stage_rows, :], stage[:, :, :]
                        )

    nc.compile()
    return nc


def build_nc_v3():
    """v2 + startup + engine-balance fixes:

    - P1 plane shifted by +2 columns (not +1): pairs taps (2,0)+(2,2) in one
      DoubleRow matmul; tap (2,1) is the single. The +2 shift keeps the P1
      fill 2-byte aligned, so it can be a CONTIGUOUS uint16 copy (bitcast):
      DVE gets its 2x mode (~0.5us/14-row piece) and Pool copies stop being
      strided-slow.
    - Weight load + prep issued first (Sync trigger order: weights, then
      image-0 x loads) and the wprep pool stays open all kernel, so no
      SBUF-reuse barrier serializes x loads behind weight transposes.
    - Store triggers on gpsimd (own queue; ACT queue head never blocks on a
      stage buffer).
    """
    FP8 = mybir.dt.float8e4
    U16 = mybir.dt.uint16
    pitch = RP
    nrows = HP + 1  # 115
    stage_rows = N_STAGEROWS
    plane = nrows * pitch  # elements between P0 and P1

    nc = bacc.Bacc(
        "TRN2", target_bir_lowering=False, debug=False, num_devices=N_CORES
    )
    x = nc.declare_dram_parameter("x", [BL, C, H, W], F32, isOutput=False)
    w = nc.declare_dram_parameter("weight", [C, C, 3, 3], F32, isOutput=False)
    y = nc.declare_dram_parameter("y", [BL, C, H, W], BF16, isOutput=True)

    with tile.TileContext(nc) as tc:
        with (
            tc.tile_pool(name="consts", bufs=1) as consts,
            tc.tile_pool(name="psum", bufs=1, space="PSUM") as psum_pool,
            tc.tile_pool(name="wprep", bufs=1) as wp,
            tc.tile_pool(name="raw", bufs=1) as raw_pool,
            tc.tile_pool(name="xpad", bufs=1) as xpad_pool,
            tc.tile_pool(name="stage", bufs=3) as stage_pool,
        ):
            # ---- weight load trigger first, then image-0 x load triggers ----
            wf = wp.tile([C, C, 3, 3], F32)
            nc.sync.dma_start(wf[:, :, :, :], w[:, :, :, :])
            load_plan0 = [14, 14, 28, 28, 28]
            raws0 = []
            r0 = 0
            for rows in load_plan0:
                raw = raw_pool.tile([C, N_LOADROWS, W], F32, tag="raw", bufs=4)
                nc.sync.dma_start(raw[:, :rows, :], x[0][:, r0 : r0 + rows, :])
                raws0.append((r0, rows, raw))
                r0 += rows

            # ---- weight prep ----
            wdr = consts.tile([C, 3, 2, C], FP8)  # pairs (0,kw)+(1,kw)
            wp2 = consts.tile([C, 2, C], FP8)  # pair (2,0)+(2,2) via planes
            w21 = consts.tile([C, C], FP8)  # tap (2,1) single
            scale = consts.tile([C, 1], F32)
            identity = consts.tile([C, C], BF16)
            make_identity(nc, identity)
            wabs = wp.tile([C, C, 3, 3], F32)
            ssum = wp.tile([C, 1], F32)
            nc.scalar.activation(
                wabs[:, :, :, :],
                wf[:, :, :, :],
                mybir.ActivationFunctionType.Abs,
                accum_out=ssum[:, :],
            )
            nc.scalar.mul(scale[:, :], ssum[:, :], 1.0 / (C * 9))
            wsign = wp.tile([C, C, 3, 3], BF16)
            nc.scalar.sign(wsign[:, :, :, :], wf[:, :, :, :])
            for t, (kh, kw) in enumerate(TAPS):
                pst = psum_pool.tile([C, C], BF16, tag="pst", bufs=2)
                nc.tensor.transpose(pst[:, :], wsign[:, :, kh, kw], identity[:, :])
                if kh < 2:
                    dst = wdr[:, kw, kh, :]
                elif kw == 0:
                    dst = wp2[:, 0, :]
                elif kw == 2:
                    dst = wp2[:, 1, :]
                else:
                    dst = w21[:, :]
                nc.vector.tensor_copy(dst, pst[:, :])

            # ---- persistent padded sign planes, double-buffered over images.
            # P0[r, 1+c] = sign(x[r-1, c]); P1[r, c] = P0[r, c+2]. ----
            xpads = []
            for k in range(2):
                xp = xpad_pool.tile(
                    [C, 2, nrows, pitch], FP8, tag=f"xpad{k}", name=f"xpad{k}"
                )
                xpads.append(xp)
                nc.gpsimd.memset(xp[:, 0, 0, :], 0.0)
                nc.gpsimd.memset(xp[:, 0, HP - 1 :, :], 0.0)
                nc.gpsimd.memset(xp[:, 0, :, W + 1 : pitch], 0.0)
                nc.gpsimd.memset(xp[:, 0, :, 0], 0.0)
                nc.gpsimd.memset(xp[:, 1, 0:2, :], 0.0)
                nc.gpsimd.memset(xp[:, 1, HP - 1 :, :], 0.0)
                nc.gpsimd.memset(xp[:, 1, :, W - 1 : pitch], 0.0)

            # P1-piece engine assignment (8 x 14-row pieces per image)
            P1_ENG = ["dve", "pool", "pool", "dve", "act", "pool", "pool", "act"]

            for n in range(BL):
                xim = x[n]
                yim = y[n]
                xpad = xpads[n % 2]
                xflat = xpad.rearrange("p a r c -> p (a r c)")
                if n == 0:
                    loads = raws0
                else:
                    loads = []
                    r0 = 0
                    for rows in [N_LOADROWS] * (H // N_LOADROWS):
                        raw = raw_pool.tile(
                            [C, N_LOADROWS, W], F32, tag="raw", bufs=4
                        )
                        nc.sync.dma_start(
                            raw[:, :rows, :], xim[:, r0 : r0 + rows, :]
                        )
                        loads.append((r0, rows, raw))
                        r0 += rows
                for r0, rows, raw in loads:
                    for a in range(0, rows, N_SIGNROWS):
                        rr = r0 + a + 1
                        piece = (r0 + a) // N_SIGNROWS
                        nc.scalar.sign(
                            xpad[:, 0, rr : rr + N_SIGNROWS, 1 : 1 + W],
                            raw[:, a : a + N_SIGNROWS, :],
                        )
                        eng = P1_ENG[piece]
                        if eng == "act":
                            # P1[r, 0:111] = sign(x[r-1, 1:112]); col 111+ is
                            # pre-zeroed (true pad)
                            nc.scalar.sign(
                                xpad[:, 1, rr : rr + N_SIGNROWS, 0 : W - 1],
                                raw[:, a : a + N_SIGNROWS, 1:W],
                            )
                        else:
                            # contiguous uint16 copy of the whole 14x128 strip,
                            # shifted 2 fp8 elements: P1[r,c] = P0[r,c+2]
                            dst = xflat[
                                :, plane + rr * pitch : plane + (rr + 14) * pitch
                            ].bitcast(U16)
                            src = xflat[
                                :, rr * pitch + 2 : (rr + 14) * pitch + 2
                            ].bitcast(U16)
                            if eng == "pool":
                                nc.gpsimd.tensor_copy(dst, src)
                            else:
                                nc.vector.tensor_copy(dst, src)

                for s0 in range(0, H, stage_rows):
                    stage = stage_pool.tile([C, stage_rows, W], BF16, tag="stage")
                    for j in range(0, stage_rows, N_ROWCHUNK):
                        h0 = s0 + j
                        NF = N_ROWCHUNK * pitch
                        ps = psum_pool.tile([C, NF], F32, tag="ps", bufs=6)
                        for kw in range(3):
                            base = xpad[:, 0, h0, kw]
                            rhs = bass.AP(
                                tensor=base.tensor,
                                offset=base.offset,
                                ap=[base.ap[0], [pitch, 2], [1, NF]],
                            )
                            nc.tensor.matmul(
                                ps[:, :],
                                wdr[:, kw, :, :],
                                rhs,
                                start=(kw == 0),
                                stop=False,
                                perf_mode=mybir.MatmulPerfMode.DoubleRow,
                            )
                        # taps (2,0)+(2,2) fused across planes
                        base = xpad[:, 0, h0 + 2, 0]
                        rhs = bass.AP(
                            tensor=base.tensor,
                            offset=base.offset,
                            ap=[base.ap[0], [plane, 2], [1, NF]],
                        )
                        nc.tensor.matmul(
                            ps[:, :],
                            wp2[:, :, :],
                            rhs,
                            start=False,
                            stop=False,
                            perf_mode=mybir.MatmulPerfMode.DoubleRow,
                        )
                        # tap (2,1): only the 448 useful columns
                        ps_rows = ps.rearrange("p (a b) -> p a b", b=pitch)[
                            :, :, 0:W
                        ]
                        base = xpad[:, 0, h0 + 2, 1]
                        rhs = bass.AP(
                            tensor=base.tensor,
                            offset=base.offset,
                            ap=[base.ap[0], [pitch, N_ROWCHUNK], [1, W]],
                        )
                        nc.tensor.matmul(
                            ps_rows, w21[:, :], rhs, start=False, stop=True
                        )
                        nc.vector.tensor_scalar_mul(
                            stage[:, j : j + N_ROWCHUNK, :], ps_rows, scale[:, :]
                        )
                    if n == BL - 1 and s0 == H - stage_rows:
                        hs = stage_rows // 2
                        nc.gpsimd.dma_start(
                            yim[:, s0 : s0 + hs, :], stage[:, :hs, :]
                        )
                        nc.gpsimd.dma_start(
                            yim[:, s0 + hs : s0 + stage_rows, :], stage[:, hs:, :]
                        )
                    else:
                        nc.gpsimd.dma_start(
                            yim[:, s0 : s0 + stage_rows, :], stage[:, :, :]
                        )

    nc.compile()
    return nc


def build_nc_v2():
    """fp8dr5 matmul scheme + three throughput changes:

    1. Output in bf16 (tolerance is 2e-2; bf16 rounding is ~2e-3): halves
       store HBM traffic, so total DMA drops from ~52 MB to ~38.8 MB/core
       (the ~358 GB/s per-core HBM limit was the #1 bottleneck).
    2. Engine rebalance: ACT was 105us busy (sign P0 + sign P1). Now the
       shifted P1 plane is filled 4/8 by Pool tensor_copy, 2/8 by ACT sign,
       2/8 by DVE copy. Output DMA triggers move from Pool(SWDGE) to the
       ACT HWDGE ring, freeing Pool for the copies.
    3. Tensor: single-tap matmul streams N=448 (3-level AP) instead of 512;
       DR matmuls stay 512 (they are LDWEIGHTS-bound anyway).
    """
    FP8 = mybir.dt.float8e4
    pitch = RP
    nrows = HP + 1  # 115: one dummy row absorbs the DR 2-element overrun
    stage_rows = N_STAGEROWS

    nc = bacc.Bacc(
        "TRN2", target_bir_lowering=False, debug=False, num_devices=N_CORES
    )
    x = nc.declare_dram_parameter("x", [BL, C, H, W], F32, isOutput=False)
    w = nc.declare_dram_parameter("weight", [C, C, 3, 3], F32, isOutput=False)
    y = nc.declare_dram_parameter("y", [BL, C, H, W], BF16, isOutput=True)

    with tile.TileContext(nc) as tc:
        with (
            tc.tile_pool(name="consts", bufs=1) as consts,
            tc.tile_pool(name="psum", bufs=1, space="PSUM") as psum_pool,
            tc.tile_pool(name="raw", bufs=1) as raw_pool,
            tc.tile_pool(name="xpad", bufs=1) as xpad_pool,
            tc.tile_pool(name="stage", bufs=3) as stage_pool,
        ):
            # ---- image-0 input loads issued before weight prep so the input
            # stream (the long pole at startup) begins immediately. First two
            # loads are 14 rows so the first Sign can start sooner.
            load_plan0 = [14, 14, 28, 28, 28]
            raws0 = []
            r0 = 0
            for rows in load_plan0:
                raw = raw_pool.tile([C, N_LOADROWS, W], F32, tag="raw", bufs=4)
                nc.sync.dma_start(raw[:, :rows, :], x[0][:, r0 : r0 + rows, :])
                raws0.append((r0, rows, raw))
                r0 += rows

            # ---- weight prep: scale[o], DR tap-pair tiles, kh=2 tiles ----
            wdr = consts.tile([C, 3, 2, C], FP8)  # pairs (0,kw)+(1,kw)
            wp2 = consts.tile([C, 2, C], FP8)  # pair (2,0)+(2,1) via planes
            w22 = consts.tile([C, C], FP8)  # tap (2,2)
            scale = consts.tile([C, 1], F32)
            identity = consts.tile([C, C], BF16)
            make_identity(nc, identity)
            with tc.tile_pool(name="wprep", bufs=1) as wp:
                wf = wp.tile([C, C, 3, 3], F32)
                nc.sync.dma_start(wf[:, :, :, :], w[:, :, :, :])
                wabs = wp.tile([C, C, 3, 3], F32)
                ssum = wp.tile([C, 1], F32)
                nc.scalar.activation(
                    wabs[:, :, :, :],
                    wf[:, :, :, :],
                    mybir.ActivationFunctionType.Abs,
                    accum_out=ssum[:, :],
                )
                nc.scalar.mul(scale[:, :], ssum[:, :], 1.0 / (C * 9))
                wsign = wp.tile([C, C, 3, 3], BF16)
                nc.scalar.sign(wsign[:, :, :, :], wf[:, :, :, :])
                for t, (kh, kw) in enumerate(TAPS):
                    pst = psum_pool.tile([C, C], BF16, tag="pst", bufs=2)
                    nc.tensor.transpose(pst[:, :], wsign[:, :, kh, kw], identity[:, :])
                    if kh < 2:
                        dst = wdr[:, kw, kh, :]
                    elif kw < 2:
                        dst = wp2[:, kw, :]
                    else:
                        dst = w22[:, :]
                    nc.vector.tensor_copy(dst, pst[:, :])

            # ---- persistent padded sign planes, double-buffered over images.
            # P0[r, 1+c] = sign(x[r-1, c]); P1[r, c] = P0[r, c+1]. Borders and
            # garbage cells zeroed once (interiors rewritten per image).
            xpads = []
            for k in range(2):
                xp = xpad_pool.tile(
                    [C, 2, nrows, pitch], FP8, tag=f"xpad{k}", name=f"xpad{k}"
                )
                xpads.append(xp)
                nc.gpsimd.memset(xp[:, 0, 0, :], 0.0)
                nc.gpsimd.memset(xp[:, 0, HP - 1 :, :], 0.0)
                nc.gpsimd.memset(xp[:, 0, :, W + 1 : pitch], 0.0)
                nc.gpsimd.memset(xp[:, 0, :, 0], 0.0)
                nc.gpsimd.memset(xp[:, 1, 0:2, :], 0.0)
                nc.gpsimd.memset(xp[:, 1, HP - 1 :, :], 0.0)
                nc.gpsimd.memset(xp[:, 1, :, W:pitch], 0.0)

            # P1-piece engine assignment by 14-row piece index (8 per image):
            # Pool copies most of it; ACT signs two pieces straight from raw;
            # DVE (busy with evacuation) takes two.
            P1_ENG = ["pool", "act", "pool", "dve", "pool", "act", "pool", "dve"]

            for n in range(BL):
                xim = x[n]
                yim = y[n]
                xpad = xpads[n % 2]
                if n == 0:
                    loads = raws0
                else:
                    loads = []
                    r0 = 0
                    for rows in [N_LOADROWS] * (H // N_LOADROWS):
                        raw = raw_pool.tile(
                            [C, N_LOADROWS, W], F32, tag="raw", bufs=4
                        )
                        nc.sync.dma_start(
                            raw[:, :rows, :], xim[:, r0 : r0 + rows, :]
                        )
                        loads.append((r0, rows, raw))
                        r0 += rows
                for r0, rows, raw in loads:
                    for a in range(0, rows, N_SIGNROWS):
                        rr = r0 + a + 1
                        piece = (r0 + a) // N_SIGNROWS
                        nc.scalar.sign(
                            xpad[:, 0, rr : rr + N_SIGNROWS, 1 : 1 + W],
                            raw[:, a : a + N_SIGNROWS, :],
                        )
                        eng = P1_ENG[piece]
                        if eng == "act":
                            nc.scalar.sign(
                                xpad[:, 1, rr : rr + N_SIGNROWS, 0:W],
                                raw[:, a : a + N_SIGNROWS, :],
                            )
                        else:
                            src = xpad[:, 0, rr : rr + N_SIGNROWS, 1 : 1 + W]
                            dst = xpad[:, 1, rr : rr + N_SIGNROWS, 0:W]
                            if eng == "pool":
                                nc.gpsimd.tensor_copy(dst, src)
                            else:
                                nc.vector.tensor_copy(dst, src)

                for s0 in range(0, H, stage_rows):
                    stage = stage_pool.tile([C, stage_rows, W], BF16, tag="stage")
                    for j in range(0, stage_rows, N_ROWCHUNK):
                        h0 = s0 + j
                        NF = N_ROWCHUNK * pitch
                        ps = psum_pool.tile([C, NF], F32, tag="ps", bufs=6)
                        for kw in range(3):
                            # taps (0,kw)+(1,kw) fused: K=256 DoubleRow
                            base = xpad[:, 0, h0, kw]
                            rhs = bass.AP(
                                tensor=base.tensor,
                                offset=base.offset,
                                ap=[base.ap[0], [pitch, 2], [1, NF]],
                            )
                            nc.tensor.matmul(
                                ps[:, :],
                                wdr[:, kw, :, :],
                                rhs,
                                start=(kw == 0),
                                stop=False,
                                perf_mode=mybir.MatmulPerfMode.DoubleRow,
                            )
                        # taps (2,0)+(2,1) fused across planes
                        base = xpad[:, 0, h0 + 2, 0]
                        rhs = bass.AP(
                            tensor=base.tensor,
                            offset=base.offset,
                            ap=[base.ap[0], [nrows * pitch, 2], [1, NF]],
                        )
                        nc.tensor.matmul(
                            ps[:, :],
                            wp2[:, :, :],
                            rhs,
                            start=False,
                            stop=False,
                            perf_mode=mybir.MatmulPerfMode.DoubleRow,
                        )
                        # tap (2,2): stream only the 448 useful columns
                        ps_rows = ps.rearrange("p (a b) -> p a b", b=pitch)[
                            :, :, 0:W
                        ]
                        base = xpad[:, 0, h0 + 2, 2]
                        rhs = bass.AP(
                            tensor=base.tensor,
                            offset=base.offset,
                            ap=[base.ap[0], [pitch, N_ROWCHUNK], [1, W]],
                        )
                        nc.tensor.matmul(
                            ps_rows, w22[:, :], rhs, start=False, stop=True
                        )
                        nc.vector.tensor_scalar_mul(
                            stage[:, j : j + N_ROWCHUNK, :], ps_rows, scale[:, :]
                        )
                    if n == BL - 1 and s0 == H - stage_rows:
                        # split the last store so the tail drains half as long
                        hs = stage_rows // 2
                        nc.scalar.dma_start(
                            yim[:, s0 : s0 + hs, :], stage[:, :hs, :]
                        )
                        nc.scalar.dma_start(
                            yim[:, s0 + hs : s0 + stage_rows, :], stage[:, hs:, :]
                        )
                    else:
                        nc.scalar.dma_start(
                            yim[:, s0 : s0 + stage_rows, :], stage[:, :, :]
                        )

    nc.compile()
    return nc


def build_nc(variant=None):
    variant = variant or VARIANT
    fp8 = variant in ("fp8dr", "fp8dr5", "fp8dr6", "fp8dr7", "fp8dr8")
    # fp8dr5: a second, column-shifted plane P1[r,c] = P0[r,c+1] lets taps
    # (2,0)+(2,1) share one DoubleRow matmul (pair step = plane stride), so a
    # chunk needs 5 matmuls instead of 6.
    planes = variant in ("fp8dr5", "fp8dr6", "fp8dr7", "fp8dr8")
    # fp8dr6: additionally (1) leave garbage-only pad cells (whose products
    # only ever land in discarded PSUM columns) unwritten, so the first
    # matmuls don't wait on slow strided memsets; (2) alternate the P1 fill
    # between ACT Sign and a DVE shift-copy to balance engine load; (3) store
    # output in 14-row pieces to shorten the kernel tail.
    lean = variant == "fp8dr6"
    stage_rows = 16 if lean else N_STAGEROWS
    # fp8dr7: fp8dr5 scheduling, but (1) buffer-1 border memsets deferred past
    # image 0 so buffer-0 init isn't queued behind them, (2) 56-row input
    # loads for images 1..3 (better DMA efficiency; image 0 keeps 28-row loads
    # for fast pipeline fill), (3) final store split to shorten the tail.
    lean7 = variant == "fp8dr7"
    # fp8dr8: ONLY the memset deferral from fp8dr7 (loads stay 28-row)
    defer = variant in ("fp8dr7", "fp8dr8")
    FP8 = mybir.dt.float8e4
    act_dt = FP8 if fp8 else BF16
    pitch = RP if fp8 else HP

    nc = bacc.Bacc(
        "TRN2", target_bir_lowering=False, debug=False, num_devices=N_CORES
    )
    x = nc.declare_dram_parameter("x", [BL, C, H, W], F32, isOutput=False)
    w = nc.declare_dram_parameter("weight", [C, C, 3, 3], F32, isOutput=False)
    y = nc.declare_dram_parameter("y", [BL, C, H, W], F32, isOutput=True)

    with tile.TileContext(nc) as tc:
        with (
            tc.tile_pool(name="consts", bufs=1) as consts,
            tc.tile_pool(name="psum", bufs=1, space="PSUM") as psum_pool,
        ):
            # ---- weight prep: scale[o] and transposed sign-weight tiles ----
            # bf16:  lhsT[i, tap, o] for the 9 taps
            # fp8dr: wdr[i, kw, j, o] pairs taps (kh=0,kw),(kh=1,kw); w2[i, kw, o]
            #        holds the kh=2 row
            if fp8:
                wdr = consts.tile([C, 3, 2, C], FP8)
                if planes:
                    wp2 = consts.tile([C, 2, C], FP8)  # taps (2,0),(2,1)
                    w22 = consts.tile([C, C], FP8)  # tap (2,2)
                else:
                    w2 = consts.tile([C, 3, C], FP8)
            else:
                lhsT = consts.tile([C, 9, C], BF16)  # [i, tap, o]
            scale = consts.tile([C, 1], F32)
            identity = consts.tile([C, C], BF16)
            make_identity(nc, identity)
            with tc.tile_pool(name="wprep", bufs=1) as wp:
                wf = wp.tile([C, C, 3, 3], F32)
                nc.sync.dma_start(wf[:, :, :, :], w[:, :, :, :])
                wabs = wp.tile([C, C, 3, 3], F32)
                ssum = wp.tile([C, 1], F32)
                nc.scalar.activation(
                    wabs[:, :, :, :],
                    wf[:, :, :, :],
                    mybir.ActivationFunctionType.Abs,
                    accum_out=ssum[:, :],
                )
                nc.scalar.mul(scale[:, :], ssum[:, :], 1.0 / (C * 9))
                wsign = wp.tile([C, C, 3, 3], BF16)
                nc.scalar.sign(wsign[:, :, :, :], wf[:, :, :, :])
                for t, (kh, kw) in enumerate(TAPS):
                    pst = psum_pool.tile([C, C], BF16, tag="pst", bufs=2)
                    nc.tensor.transpose(pst[:, :], wsign[:, :, kh, kw], identity[:, :])
                    if fp8 and planes:
                        if kh < 2:
                            dst = wdr[:, kw, kh, :]
                        elif kw < 2:
                            dst = wp2[:, kw, :]
                        else:
                            dst = w22[:, :]
                    elif fp8:
                        dst = wdr[:, kw, kh, :] if kh < 2 else w2[:, kw, :]
                    else:
                        dst = lhsT[:, t, :]
                    # DVE, not ACT: keeps ACT free for the first image's Sign
                    nc.vector.tensor_copy(dst, pst[:, :])

            # ---- main loop over local images ----
            with (
                tc.tile_pool(name="raw", bufs=2) as raw_pool,
                tc.tile_pool(name="xpad", bufs=1) as xpad_pool,
                tc.tile_pool(name="stage", bufs=3) as stage_pool,
            ):
                # Two persistent padded buffers, manually double-buffered
                # across images. Borders are zeroed ONCE here (the interior is
                # rewritten per image, borders stay zero), so image-boundary
                # matmuls never wait on memsets queued behind output DMAs.
                # fp8dr reads whole pitch-128 rows (N=512 contiguous spans);
                # one extra dummy row absorbs the last chunk's 2-element
                # overrun, and every non-interior cell is zeroed.
                nrows = HP + 1 if fp8 else HP
                nplanes = 2 if planes else 1

                def border_memsets(xp):
                    nc.gpsimd.memset(xp[:, 0, 0, :], 0.0)
                    nc.gpsimd.memset(xp[:, 0, HP - 1 :, :], 0.0)
                    nc.gpsimd.memset(xp[:, 0, :, W + 1 : pitch], 0.0)
                    nc.gpsimd.memset(xp[:, 0, :, 0], 0.0)
                    nc.gpsimd.memset(xp[:, 1, 0:2, :], 0.0)
                    nc.gpsimd.memset(xp[:, 1, HP - 1 :, :], 0.0)
                    nc.gpsimd.memset(xp[:, 1, :, W:pitch], 0.0)

                xpads = []
                for k in range(2):
                    xp = xpad_pool.tile(
                        [C, nplanes, nrows, pitch],
                        act_dt,
                        tag=f"xpad{k}",
                        name=f"xpad{k}",
                    )
                    xpads.append(xp)
                    if defer:
                        if k == 0:
                            border_memsets(xp)
                        continue
                    nc.gpsimd.memset(xp[:, 0, 0, :], 0.0)
                    if lean:
                        # thin true-pad strips on gpsimd (fast), fat
                        # garbage-only strips on the (idle-at-start) DVE, so
                        # buffer init never gates the first matmuls
                        nc.gpsimd.memset(xp[:, 0, HP - 1 :, :], 0.0)
                        nc.gpsimd.memset(xp[:, 0, 1 : HP - 1, 0], 0.0)
                        nc.gpsimd.memset(xp[:, 0, 1 : HP - 1, W + 1], 0.0)
                        nc.gpsimd.memset(xp[:, 1, HP - 1 :, :], 0.0)
                        nc.vector.memset(xp[:, 0, 1 : HP - 1, W + 2 : pitch], 0.0)
                        nc.vector.memset(xp[:, 1, 2 : HP - 1, W : pitch], 0.0)
                    elif fp8:
                        nc.gpsimd.memset(xp[:, 0, HP - 1 :, :], 0.0)
                        nc.gpsimd.memset(xp[:, 0, :, W + 1 : pitch], 0.0)
                        nc.gpsimd.memset(xp[:, 0, :, 0], 0.0)
                        if planes:
                            nc.gpsimd.memset(xp[:, 1, 0:2, :], 0.0)
                            nc.gpsimd.memset(xp[:, 1, HP - 1 :, :], 0.0)
                            nc.gpsimd.memset(xp[:, 1, :, W:pitch], 0.0)
                    else:
                        nc.gpsimd.memset(xp[:, 0, HP - 1, :], 0.0)
                        nc.gpsimd.memset(xp[:, 0, :, HP - 1], 0.0)
                        nc.gpsimd.memset(xp[:, 0, :, 0], 0.0)
                for n in range(BL):
                    xim = x[n]  # [C, H, W]
                    yim = y[n]
                    xpad = xpads[n % 2]
                    if lean7 and n > 0:
                        load_sizes = [56, 56]
                    else:
                        load_sizes = [N_LOADROWS] * (H // N_LOADROWS)
                    raw_rows = 56 if lean7 else N_LOADROWS
                    r0 = 0
                    for rows in load_sizes:
                        raw = raw_pool.tile(
                            [C, raw_rows, W], F32, tag="raw",
                            bufs=2 if lean7 else 4,
                        )
                        nc.sync.dma_start(
                            raw[:, :rows, :], xim[:, r0 : r0 + rows, :]
                        )
                        for a in range(0, rows, N_SIGNROWS):
                            rr = r0 + a + 1
                            nc.scalar.sign(
                                xpad[:, 0, rr : rr + N_SIGNROWS, 1 : 1 + W],
                                raw[:, a : a + N_SIGNROWS, :],
                            )
                            if planes and lean and (a // N_SIGNROWS) % 2 == 1:
                                # balance engines: every other P1 piece is a
                                # DVE shift-copy of P0 instead of an ACT Sign
                                nc.vector.tensor_copy(
                                    xpad[:, 1, rr : rr + N_SIGNROWS, 0:W],
                                    xpad[:, 0, rr : rr + N_SIGNROWS, 1 : 1 + W],
                                )
                            elif planes:
                                nc.scalar.sign(
                                    xpad[:, 1, rr : rr + N_SIGNROWS, 0:W],
                                    raw[:, a : a + N_SIGNROWS, :],
                                )
                        r0 += rows
                    if defer and n == 0:
                        # buffer 1 isn't read until image 1: zero its borders
                        # only now, so buffer 0's init wasn't queued behind it
                        border_memsets(xpads[1])
                    for s0 in range(0, H, stage_rows):
                        stage = stage_pool.tile([C, stage_rows, W], F32, tag="stage")
                        for j in range(0, stage_rows, N_ROWCHUNK):
                            h0 = s0 + j
                            if fp8:
                                # full-pitch output rows: N = 4*128 = 512 fp32
                                # (one PSUM bank); cols >= 112 of each row are
                                # garbage and skipped at evacuation
                                NF = N_ROWCHUNK * pitch
                                ps = psum_pool.tile([C, NF], F32, tag="ps", bufs=6)
                                for kw in range(3):
                                    # taps (0,kw)+(1,kw) fused: K=256 DoubleRow
                                    base = xpad[:, 0, h0, kw]
                                    rhs = bass.AP(
                                        tensor=base.tensor,
                                        offset=base.offset,
                                        ap=[base.ap[0], [pitch, 2], [1, NF]],
                                    )
                                    nc.tensor.matmul(
                                        ps[:, :],
                                        wdr[:, kw, :, :],
                                        rhs,
                                        start=(kw == 0),
                                        stop=False,
                                        perf_mode=mybir.MatmulPerfMode.DoubleRow,
                                    )
                                if planes:
                                    # taps (2,0)+(2,1) fused across the P0/P1
                                    # planes (pair step = plane stride)
                                    base = xpad[:, 0, h0 + 2, 0]
                                    rhs = bass.AP(
                                        tensor=base.tensor,
                                        offset=base.offset,
                                        ap=[base.ap[0], [nrows * pitch, 2], [1, NF]],
                                    )
                                    nc.tensor.matmul(
                                        ps[:, :],
                                        wp2[:, :, :],
                                        rhs,
                                        start=False,
                                        stop=False,
                                        perf_mode=mybir.MatmulPerfMode.DoubleRow,
                                    )
                                    base = xpad[:, 0, h0 + 2, 2]
                                    rhs = bass.AP(
                                        tensor=base.tensor,
                                        offset=base.offset,
                                        ap=[base.ap[0], [1, NF]],
                                    )
                                    nc.tensor.matmul(
                                        ps[:, :],
                                        w22[:, :],
                                        rhs,
                                        start=False,
                                        stop=True,
                                    )
                                else:
                                    for kw in range(3):
                                        # tap (2,kw)
                                        base = xpad[:, 0, h0 + 2, kw]
                                        rhs = bass.AP(
                                            tensor=base.tensor,
                                            offset=base.offset,
                                            ap=[base.ap[0], [1, NF]],
                                        )
                                        nc.tensor.matmul(
                                            ps[:, :],
                                            w2[:, kw, :],
                                            rhs,
                                            start=False,
                                            stop=(kw == 2),
                                        )
                                ps_rows = ps.rearrange(
                                    "p (a b) -> p a b", b=pitch
                                )[:, :, 0:W]
                            else:
                                ps = psum_pool.tile(
                                    [C, N_ROWCHUNK, W], F32, tag="ps", bufs=6
                                )
                                for t, (kh, kw) in enumerate(TAPS):
                                    nc.tensor.matmul(
                                        ps[:, :, :],
                                        lhsT[:, t, :],
                                        xpad[
                                            :,
                                            0,
                                            h0 + kh : h0 + kh + N_ROWCHUNK,
                                            kw : kw + W,
                                        ],
                                        start=(t == 0),
                                        stop=(t == len(TAPS) - 1),
                                    )
                                ps_rows = ps[:, :, :]
                            nc.vector.tensor_scalar_mul(
                                stage[:, j : j + N_ROWCHUNK, :], ps_rows, scale[:, :]
                            )
                        if lean7 and n == BL - 1 and s0 == H - stage_rows:
                            # split the very last store so the kernel tail only
                            # waits on half the bytes
                            hs = stage_rows // 2
                            nc.gpsimd.dma_start(
                                yim[:, s0 : s0 + hs, :], stage[:, :hs, :]
                            )
                            nc.gpsimd.dma_start(
                                yim[:, s0 + hs : s0 + stage_rows, :],
                                stage[:, hs:, :],
                            )
                        else:
                            nc.gpsimd.dma_start(
                                yim[:, s0 : s0 + stage_rows, :], stage[:, :, :]
                            )

    nc.compile()
    return nc


_NC_CACHE = {}


def _get_nc(variant=None):
    variant = variant or VARIANT
    if variant not in _NC_CACHE:
        if variant == "v2":
            _NC_CACHE[variant] = build_nc_v2()
        elif variant == "v3":
            _NC_CACHE[variant] = build_nc_v3()
        elif variant in ("v4", "v5", "v6", "v7"):
            _NC_CACHE[variant] = build_nc_v4(variant)
        else:
            _NC_CACHE[variant] = build_nc(variant)
    return _NC_CACHE[variant]


def kernel(
    x: np.ndarray,
    weight: np.ndarray,
    _trace: bool = False,
    _variant: str | None = None,
    **_kw,
):
    assert x.shape == (B, C, H, W) and weight.shape == (C, C, 3, 3)
    nc = _get_nc(_variant)
    xs = np.ascontiguousarray(x, dtype=np.float32)
    wgt = np.ascontiguousarray(weight, dtype=np.float32)
    in_maps = [
        {"x": xs[i * BL : (i + 1) * BL], "weight": wgt} for i in range(N_CORES)
    ]
    res = run_bass_kernel_spmd(
        nc, in_maps, core_ids=list(range(N_CORES)), trace=_trace
    )
    out = np.concatenate(
        [np.asarray(res.results[i]["y"], dtype=np.float32) for i in range(N_CORES)],
        axis=0,
    )
    if _trace:
        kernel.last_results = res
    return out



# revision 41
# speedup vs baseline: 1.1933x; 1.1933x over previous
"""Bi-Real Net binary conv2d (3x3, pad 1, stride 1) for Trainium2, 8 NeuronCores.

Math (forward values of the reference):
    xb = sign(x)                      in {-1, 0, +1}
    scale[o] = mean_{i,kh,kw} |w[o,i,kh,kw]|
    wb = scale[o] * sign(w)
    y = conv2d_NCHW(xb, wb, pad=1)

Kernel strategy:
    - Data-parallel over batch: 32 images -> 4 per core on 8 cores.
    - Per image: DMA [128, 112*112] f32 -> SBUF, ACT Sign -> zero-padded
      bf16 buffer [128, 114, 114].
    - Conv as 9 accumulated matmuls per 4-output-row chunk:
      psum[o, 4x112] += signW_tap[i, o].T @ xpad[i, rows+kh, kw:kw+112].
      Products are +-1 in bf16 (exact); PSUM accumulates exact integers.
    - PSUM evacuation on DVE multiplies by per-channel scale[o] (fp32).
    - Outputs staged in SBUF (16 rows) and DMA'd out in ~0.9 MB chunks.
"""

import sys

sys.path.insert(0, "/opt/trn_rl_repo")

import numpy as np

import concourse.bacc as bacc
import concourse.bass as bass
import concourse.mybir as mybir
import concourse.tile as tile
from concourse.bass_utils import run_bass_kernel_spmd
from concourse.masks import make_identity

N_CORES = 8
B, C, H, W = 32, 128, 112, 112
BL = B // N_CORES  # images per core
HP = H + 2  # padded height/width (114)
TAPS = [(kh, kw) for kh in range(3) for kw in range(3)]

F32 = mybir.dt.float32
BF16 = mybir.dt.bfloat16

N_ROWCHUNK = 4  # output rows per PSUM accumulation group (<= one 2KB bank)
N_STAGEROWS = 28  # output rows per SBUF->DRAM store (must divide 112)
N_LOADROWS = 28  # input rows per DRAM->SBUF load
N_SIGNROWS = 14  # input rows per ACT Sign instruction


RP = 128  # fp8 padded-row pitch; 128 makes the DoubleRow mid-dim step %16==0

VARIANT = "v7"  # "bf16" | "fp8dr" | "fp8dr5" | "v2" | "v3" | "v4" | "v5"


def build_nc_v4(variant="v4"):
    """v3 + row-interleaved P1 plane and 4-level matmul rhs APs.

    v3's plane-pair DoubleRow matmul streamed ~1.75x slower than the
    vertical pairs (pair elements 14.7KB apart in SBUF). Interleaving
    P0/P1 rows ([C, nrows, 2, pitch]) makes every pair step 128 or 256
    bytes. The rhs then needs [pair][row][col] = 4-level APs, which also
    lets all matmuls stream only the 448 useful columns (no garbage
    columns), and PSUM chunks become contiguous 448-col banks.

    v5: startup/tail polish — image-0 P1 pieces on DVE (ACT was the
    warmup serializer), weight abs/scale on DVE (shortens the ACT chain
    to the first sign), last image staged in 14-row pieces (shorter
    drain tail).
    """
    v5 = variant in ("v5", "v6", "v7")
    v6 = variant in ("v6", "v7")
    v7 = variant == "v7"
    FP8 = mybir.dt.float8e4
    U16 = mybir.dt.uint16
    pitch = RP
    nrows = HP + 1  # 115
    rstride = 2 * pitch  # consecutive-row stride in the interleaved layout
    stage_rows = N_STAGEROWS

    nc = bacc.Bacc(
        "TRN2", target_bir_lowering=False, debug=False, num_devices=N_CORES
    )
    x = nc.declare_dram_parameter("x", [BL, C, H, W], F32, isOutput=False)
    w = nc.declare_dram_parameter("weight", [C, C, 3, 3], F32, isOutput=False)
    y = nc.declare_dram_parameter("y", [BL, C, H, W], BF16, isOutput=True)

    with tile.TileContext(nc) as tc:
        with (
            tc.tile_pool(name="consts", bufs=1) as consts,
            tc.tile_pool(name="psum", bufs=1, space="PSUM") as psum_pool,
            tc.tile_pool(name="wprep", bufs=1) as wp,
            tc.tile_pool(name="raw", bufs=1) as raw_pool,
            tc.tile_pool(name="xpad", bufs=1) as xpad_pool,
            tc.tile_pool(name="stage", bufs=3) as stage_pool,
        ):
            wf = wp.tile([C, C, 3, 3], F32)
            nc.sync.dma_start(wf[:, :, :, :], w[:, :, :, :])
            load_plan0 = [14, 14, 28, 28, 28]
            raws0 = []
            r0 = 0
            for rows in load_plan0:
                raw = raw_pool.tile([C, N_LOADROWS, W], F32, tag="raw", bufs=4)
                nc.sync.dma_start(raw[:, :rows, :], x[0][:, r0 : r0 + rows, :])
                raws0.append((r0, rows, raw))
                r0 += rows

            wdr = consts.tile([C, 3, 2, C], FP8)
            wp2 = consts.tile([C, 2, C], FP8)
            if v7:
                # tap (2,1) paired with a zero weight row: the 5th pass is a
                # DoubleRow like the others (uniform DR chunks issue cleanly)
                w21 = consts.tile([C, 2, C], FP8)
                nc.vector.memset(w21[:, 1, :], 0.0)
            else:
                w21 = consts.tile([C, C], FP8)
            scale = consts.tile([C, 1], F32)
            identity = consts.tile([C, C], BF16)
            make_identity(nc, identity)
            wabs = wp.tile([C, C, 3, 3], F32)
            ssum = wp.tile([C, 1], F32)
            # wsign split by kh so the first transposes start ~0.8us sooner;
            # abs/scale deferred (only needed by the first evacuation) so
            # ACT reaches image-0 signs earlier
            wsign = wp.tile([C, C, 3, 3], BF16)
            if v6:
                for kh in range(3):
                    nc.scalar.sign(wsign[:, :, kh, :], wf[:, :, kh, :])
            else:
                nc.scalar.sign(wsign[:, :, :, :], wf[:, :, :, :])

            def emit_weight_scale():
                nc.scalar.activation(
                    wabs[:, :, :, :],
                    wf[:, :, :, :],
                    mybir.ActivationFunctionType.Abs,
                    accum_out=ssum[:, :],
                )
                nc.scalar.mul(scale[:, :], ssum[:, :], 1.0 / (C * 9))

            if not v6:
                emit_weight_scale()
            for t, (kh, kw) in enumerate(TAPS):
                pst = psum_pool.tile([C, C], BF16, tag="pst", bufs=2)
                nc.tensor.transpose(pst[:, :], wsign[:, :, kh, kw], identity[:, :])
                if kh < 2:
                    dst = wdr[:, kw, kh, :]
                elif kw == 0:
                    dst = wp2[:, 0, :]
                elif kw == 2:
                    dst = wp2[:, 1, :]
                elif v7:
                    dst = w21[:, 0, :]
                else:
                    dst = w21[:, :]
                nc.vector.tensor_copy(dst, pst[:, :])

            # interleaved planes: [C, row, {P0,P1}, pitch]
            # P0[r, 1+c] = sign(x[r-1, c]); P1[r, c] = P0[r, c+2]
            xpads = []
            for k in range(2):
                xp = xpad_pool.tile(
                    [C, nrows, 2, pitch], FP8, tag=f"xpad{k}", name=f"xpad{k}"
                )
                xpads.append(xp)
                nc.gpsimd.memset(xp[:, 0, 0, :], 0.0)
                nc.gpsimd.memset(xp[:, HP - 1 :, 0, :], 0.0)
                nc.gpsimd.memset(xp[:, :, 0, W + 1 : pitch], 0.0)
                nc.gpsimd.memset(xp[:, :, 0, 0], 0.0)
                nc.gpsimd.memset(xp[:, 0:2, 1, :], 0.0)
                nc.gpsimd.memset(xp[:, HP - 1 :, 1, :], 0.0)
                nc.gpsimd.memset(xp[:, :, 1, W - 1 : pitch], 0.0)

            # piece 0 pre-signable early by ACT; early pieces on DVE (short
            # queue coupling); late pieces on the lazy Pool engine
            P1_ENG = ["act", "dve", "dve", "dve", "pool", "pool", "pool", "dve"]
            # image 0: ACT is the warmup serializer (wsign + P0 signs), so
            # its P1 piece goes to DVE as well
            P1_ENG0 = ["dve", "dve", "dve", "dve", "pool", "pool", "pool", "dve"]

            for n in range(BL):
                xim = x[n]
                yim = y[n]
                xpad = xpads[n % 2]
                xflat = xpad.rearrange("p r a c -> p (r a c)")
                if n == 0:
                    loads = raws0
                else:
                    loads = []
                    r0 = 0
                    for rows in [N_LOADROWS] * (H // N_LOADROWS):
                        raw = raw_pool.tile(
                            [C, N_LOADROWS, W], F32, tag="raw", bufs=4
                        )
                        nc.sync.dma_start(
                            raw[:, :rows, :], xim[:, r0 : r0 + rows, :]
                        )
                        loads.append((r0, rows, raw))
                        r0 += rows
                for r0, rows, raw in loads:
                    for a in range(0, rows, N_SIGNROWS):
                        rr = r0 + a + 1
                        piece = (r0 + a) // N_SIGNROWS
                        nc.scalar.sign(
                            xpad[:, rr : rr + N_SIGNROWS, 0, 1 : 1 + W],
                            raw[:, a : a + N_SIGNROWS, :],
                        )
                        if v6 and n == 0 and piece == 1 and a == 0:
                            # scale is first needed once 6 PSUM banks fill
                            # (~6 chunks in); emitting here keeps both the
                            # piece-1 sign and the first evacuation unblocked
                            emit_weight_scale()
                        eng = (P1_ENG0 if (v5 and n == 0) else P1_ENG)[piece]
                        if eng == "act":
                            nc.scalar.sign(
                                xpad[:, rr : rr + N_SIGNROWS, 1, 0 : W - 1],
                                raw[:, a : a + N_SIGNROWS, 1:W],
                            )
                        else:
                            # 14 rows of u16: dst = P1 rows, src = P0 rows
                            # shifted +2 fp8 elements (1 u16 element)
                            db = (rr * 2 + 1) * pitch // 2  # u16 offset
                            sb = rr * 2 * pitch // 2 + 1
                            base = xflat[:, 0:2].bitcast(U16)
                            dst = bass.AP(
                                tensor=base.tensor,
                                offset=base.offset + db,
                                ap=[base.ap[0], [pitch, N_SIGNROWS], [1, pitch // 2]],
                            )
                            src = bass.AP(
                                tensor=base.tensor,
                                offset=base.offset + sb,
                                ap=[base.ap[0], [pitch, N_SIGNROWS], [1, pitch // 2]],
                            )
                            if eng == "pool":
                                nc.gpsimd.tensor_copy(dst, src)
                            else:
                                nc.vector.tensor_copy(dst, src)

                for s0 in range(0, H, stage_rows):
                    stage = stage_pool.tile([C, stage_rows, W], BF16, tag="stage")
                    for j in range(0, stage_rows, N_ROWCHUNK):
                        h0 = s0 + j
                        NF = N_ROWCHUNK * W  # 448
                        ps = psum_pool.tile([C, NF], F32, tag="ps", bufs=6)

                        def mm_vert(kw, start, stop):
                            base = xpad[:, h0, 0, kw]
                            rhs = bass.AP(
                                tensor=base.tensor,
                                offset=base.offset,
                                ap=[
                                    base.ap[0],
                                    [rstride, 2],
                                    [rstride, N_ROWCHUNK],
                                    [1, W],
                                ],
                            )
                            nc.tensor.matmul(
                                ps[:, :],
                                wdr[:, kw, :, :],
                                rhs,
                                start=start,
                                stop=stop,
                                perf_mode=mybir.MatmulPerfMode.DoubleRow,
                            )

                        def mm_plane(start, stop):
                            # taps (2,0)+(2,2): pair step 128 (adjacent planes)
                            base = xpad[:, h0 + 2, 0, 0]
                            rhs = bass.AP(
                                tensor=base.tensor,
                                offset=base.offset,
                                ap=[
                                    base.ap[0],
                                    [pitch, 2],
                                    [rstride, N_ROWCHUNK],
                                    [1, W],
                                ],
                            )
                            nc.tensor.matmul(
                                ps[:, :],
                                wp2[:, :, :],
                                rhs,
                                start=start,
                                stop=stop,
                                perf_mode=mybir.MatmulPerfMode.DoubleRow,
                            )

                        def mm_single(start, stop):
                            # tap (2,1)
                            base = xpad[:, h0 + 2, 0, 1]
                            if v7:
                                rhs = bass.AP(
                                    tensor=base.tensor,
                                    offset=base.offset,
                                    ap=[
                                        base.ap[0],
                                        [rstride, 2],
                                        [rstride, N_ROWCHUNK],
                                        [1, W],
                                    ],
                                )
                                nc.tensor.matmul(
                                    ps[:, :],
                                    w21[:, :, :],
                                    rhs,
                                    start=start,
                                    stop=stop,
                                    perf_mode=mybir.MatmulPerfMode.DoubleRow,
                                )
                            else:
                                rhs = bass.AP(
                                    tensor=base.tensor,
                                    offset=base.offset,
                                    ap=[base.ap[0], [rstride, N_ROWCHUNK], [1, W]],
                                )
                                nc.tensor.matmul(
                                    ps[:, :], w21[:, :], rhs, start=start, stop=stop
                                )

                        mm_vert(0, True, False)
                        mm_vert(1, False, False)
                        mm_vert(2, False, False)
                        mm_plane(False, False)
                        mm_single(False, True)
                        nc.vector.tensor_scalar_mul(
                            stage[:, j : j + N_ROWCHUNK, :], ps[:, :], scale[:, :]
                        )
                    if v7 and n == BL - 1:
                        # last image's stores on the (idle by now) ACT HWDGE
                        # ring: the SWDGE gpsimd drain was 3.6us of pure tail
                        if s0 == H - stage_rows:
                            for j in range(0, stage_rows, stage_rows // 2):
                                nc.scalar.dma_start(
                                    yim[:, s0 + j : s0 + j + stage_rows // 2, :],
                                    stage[:, j : j + stage_rows // 2, :],
                                )
                        else:
                            nc.scalar.dma_start(
                                yim[:, s0 : s0 + stage_rows, :], stage[:, :, :]
                            )
                    elif n == BL - 1 and s0 == H - stage_rows:
                        if v5:
                            # drain the tail in 4-row pieces so the last
                            # transfer is tiny
                            for j in range(0, stage_rows, N_ROWCHUNK):
                                nc.gpsimd.dma_start(
                                    yim[:, s0 + j : s0 + j + N_ROWCHUNK, :],
                                    stage[:, j : j + N_ROWCHUNK, :],
                                )
                        else:
                            hs = stage_rows // 2
                            nc.gpsimd.dma_start(
                                yim[:, s0 : s0 + hs, :], stage[:, :hs, :]
                            )
                            nc.gpsimd.dma_start(
                                yim[:, s0 + hs : s0 + stage_rows, :],
                                stage[:, hs:, :],
                            )
                    else:
                        nc.gpsimd.dma_start(
                            yim[:, s0 : s0 + stage_rows, :], stage[:, :, :]
                        )

    nc.compile()
    return nc


def build_nc_v3():
    """v2 + startup + engine-balance fixes:

    - P1 plane shifted by +2 columns (not +1): pairs taps (2,0)+(2,2) in one
      DoubleRow matmul; tap (2,1) is the single. The +2 shift keeps the P1
      fill 2-byte aligned, so it can be a CONTIGUOUS uint16 copy (bitcast):
      DVE gets its 2x mode (~0.5us/14-row piece) and Pool copies stop being
      strided-slow.
    - Weight load + prep issued first (Sync trigger order: weights, then
      image-0 x loads) and the wprep pool stays open all kernel, so no
      SBUF-reuse barrier serializes x loads behind weight transposes.
    - Store triggers on gpsimd (own queue; ACT queue head never blocks on a
      stage buffer).
    """
    FP8 = mybir.dt.float8e4
    U16 = mybir.dt.uint16
    pitch = RP
    nrows = HP + 1  # 115
    stage_rows = N_STAGEROWS
    plane = nrows * pitch  # elements between P0 and P1

    nc = bacc.Bacc(
        "TRN2", target_bir_lowering=False, debug=False, num_devices=N_CORES
    )
    x = nc.declare_dram_parameter("x", [BL, C, H, W], F32, isOutput=False)
    w = nc.declare_dram_parameter("weight", [C, C, 3, 3], F32, isOutput=False)
    y = nc.declare_dram_parameter("y", [BL, C, H, W], BF16, isOutput=True)

    with tile.TileContext(nc) as tc:
        with (
            tc.tile_pool(name="consts", bufs=1) as consts,
            tc.tile_pool(name="psum", bufs=1, space="PSUM") as psum_pool,
            tc.tile_pool(name="wprep", bufs=1) as wp,
            tc.tile_pool(name="raw", bufs=1) as raw_pool,
            tc.tile_pool(name="xpad", bufs=1) as xpad_pool,
            tc.tile_pool(name="stage", bufs=3) as stage_pool,
        ):
            # ---- weight load trigger first, then image-0 x load triggers ----
            wf = wp.tile([C, C, 3, 3], F32)
            nc.sync.dma_start(wf[:, :, :, :], w[:, :, :, :])
            load_plan0 = [14, 14, 28, 28, 28]
            raws0 = []
            r0 = 0
            for rows in load_plan0:
                raw = raw_pool.tile([C, N_LOADROWS, W], F32, tag="raw", bufs=4)
                nc.sync.dma_start(raw[:, :rows, :], x[0][:, r0 : r0 + rows, :])
                raws0.append((r0, rows, raw))
                r0 += rows

            # ---- weight prep ----
            wdr = consts.tile([C, 3, 2, C], FP8)  # pairs (0,kw)+(1,kw)
            wp2 = consts.tile([C, 2, C], FP8)  # pair (2,0)+(2,2) via planes
            w21 = consts.tile([C, C], FP8)  # tap (2,1) single
            scale = consts.tile([C, 1], F32)
            identity = consts.tile([C, C], BF16)
            make_identity(nc, identity)
            wabs = wp.tile([C, C, 3, 3], F32)
            ssum = wp.tile([C, 1], F32)
            nc.scalar.activation(
                wabs[:, :, :, :],
                wf[:, :, :, :],
                mybir.ActivationFunctionType.Abs,
                accum_out=ssum[:, :],
            )
            nc.scalar.mul(scale[:, :], ssum[:, :], 1.0 / (C * 9))
            wsign = wp.tile([C, C, 3, 3], BF16)
            nc.scalar.sign(wsign[:, :, :, :], wf[:, :, :, :])
            for t, (kh, kw) in enumerate(TAPS):
                pst = psum_pool.tile([C, C], BF16, tag="pst", bufs=2)
                nc.tensor.transpose(pst[:, :], wsign[:, :, kh, kw], identity[:, :])
                if kh < 2:
                    dst = wdr[:, kw, kh, :]
                elif kw == 0:
                    dst = wp2[:, 0, :]
                elif kw == 2:
                    dst = wp2[:, 1, :]
                else:
                    dst = w21[:, :]
                nc.vector.tensor_copy(dst, pst[:, :])

            # ---- persistent padded sign planes, double-buffered over images.
            # P0[r, 1+c] = sign(x[r-1, c]); P1[r, c] = P0[r, c+2]. ----
            xpads = []
            for k in range(2):
                xp = xpad_pool.tile(
                    [C, 2, nrows, pitch], FP8, tag=f"xpad{k}", name=f"xpad{k}"
                )
                xpads.append(xp)
                nc.gpsimd.memset(xp[:, 0, 0, :], 0.0)
                nc.gpsimd.memset(xp[:, 0, HP - 1 :, :], 0.0)
                nc.gpsimd.memset(xp[:, 0, :, W + 1 : pitch], 0.0)
                nc.gpsimd.memset(xp[:, 0, :, 0], 0.0)
                nc.gpsimd.memset(xp[:, 1, 0:2, :], 0.0)
                nc.gpsimd.memset(xp[:, 1, HP - 1 :, :], 0.0)
                nc.gpsimd.memset(xp[:, 1, :, W - 1 : pitch], 0.0)

            # P1-piece engine assignment (8 x 14-row pieces per image)
            P1_ENG = ["dve", "pool", "pool", "dve", "act", "pool", "pool", "act"]

            for n in range(BL):
                xim = x[n]
                yim = y[n]
                xpad = xpads[n % 2]
                xflat = xpad.rearrange("p a r c -> p (a r c)")
                if n == 0:
                    loads = raws0
                else:
                    loads = []
                    r0 = 0
                    for rows in [N_LOADROWS] * (H // N_LOADROWS):
                        raw = raw_pool.tile(
                            [C, N_LOADROWS, W], F32, tag="raw", bufs=4
                        )
                        nc.sync.dma_start(
                            raw[:, :rows, :], xim[:, r0 : r0 + rows, :]
                        )
                        loads.append((r0, rows, raw))
                        r0 += rows
                for r0, rows, raw in loads:
                    for a in range(0, rows, N_SIGNROWS):
                        rr = r0 + a + 1
                        piece = (r0 + a) // N_SIGNROWS
                        nc.scalar.sign(
                            xpad[:, 0, rr : rr + N_SIGNROWS, 1 : 1 + W],
                            raw[:, a : a + N_SIGNROWS, :],
                        )
                        eng = P1_ENG[piece]
                        if eng == "act":
                            # P1[r, 0:111] = sign(x[r-1, 1:112]); col 111+ is
                            # pre-zeroed (true pad)
                            nc.scalar.sign(
                                xpad[:, 1, rr : rr + N_SIGNROWS, 0 : W - 1],
                                raw[:, a : a + N_SIGNROWS, 1:W],
                            )
                        else:
                            # contiguous uint16 copy of the whole 14x128 strip,
                            # shifted 2 fp8 elements: P1[r,c] = P0[r,c+2]
                            dst = xflat[
                                :, plane + rr * pitch : plane + (rr + 14) * pitch
                            ].bitcast(U16)
                            src = xflat[
                                :, rr * pitch + 2 : (rr + 14) * pitch + 2
                            ].bitcast(U16)
                            if eng == "pool":
                                nc.gpsimd.tensor_copy(dst, src)
                            else:
                                nc.vector.tensor_copy(dst, src)

                for s0 in range(0, H, stage_rows):
                    stage = stage_pool.tile([C, stage_rows, W], BF16, tag="stage")
                    for j in range(0, stage_rows, N_ROWCHUNK):
                        h0 = s0 + j
                        NF = N_ROWCHUNK * pitch
                        ps = psum_pool.tile([C, NF], F32, tag="ps", bufs=6)
                        for kw in range(3):
                            base = xpad[:, 0, h0, kw]
                            rhs = bass.AP(
                                tensor=base.tensor,
                                offset=base.offset,
                                ap=[base.ap[0], [pitch, 2], [1, NF]],
                            )
                            nc.tensor.matmul(
                                ps[:, :],
                                wdr[:, kw, :, :],
                                rhs,
                                start=(kw == 0),
                                stop=False,
                                perf_mode=mybir.MatmulPerfMode.DoubleRow,
                            )
                        # taps (2,0)+(2,2) fused across planes
                        base = xpad[:, 0, h0 + 2, 0]
                        rhs = bass.AP(
                            tensor=base.tensor,
                            offset=base.offset,
                            ap=[base.ap[0], [plane, 2], [1, NF]],
                        )
                        nc.tensor.matmul(
                            ps[:, :],
                            wp2[:, :, :],
                            rhs,
                            start=False,
                            stop=False,
                            perf_mode=mybir.MatmulPerfMode.DoubleRow,
                        )
                        # tap (2,1): only the 448 useful columns
                        ps_rows = ps.rearrange("p (a b) -> p a b", b=pitch)[
                            :, :, 0:W
                        ]
                        base = xpad[:, 0, h0 + 2, 1]
                        rhs = bass.AP(
                            tensor=base.tensor,
                            offset=base.offset,
                            ap=[base.ap[0], [pitch, N_ROWCHUNK], [1, W]],
                        )
                        nc.tensor.matmul(
                            ps_rows, w21[:, :], rhs, start=False, stop=True
                        )
                        nc.vector.tensor_scalar_mul(
                            stage[:, j : j + N_ROWCHUNK, :], ps_rows, scale[:, :]
                        )
                    if n == BL - 1 and s0 == H - stage_rows:
                        hs = stage_rows // 2
                        nc.gpsimd.dma_start(
                            yim[:, s0 : s0 + hs, :], stage[:, :hs, :]
                        )
                        nc.gpsimd.dma_start(
                            yim[:, s0 + hs : s0 + stage_rows, :], stage[:, hs:, :]
                        )
                    else:
                        nc.gpsimd.dma_start(
                            yim[:, s0 : s0 + stage_rows, :], stage[:, :, :]
                        )

    nc.compile()
    return nc


def build_nc_v2():
    """fp8dr5 matmul scheme + three throughput changes:

    1. Output in bf16 (tolerance is 2e-2; bf16 rounding is ~2e-3): halves
       store HBM traffic, so total DMA drops from ~52 MB to ~38.8 MB/core
       (the ~358 GB/s per-core HBM limit was the #1 bottleneck).
    2. Engine rebalance: ACT was 105us busy (sign P0 + sign P1). Now the
       shifted P1 plane is filled 4/8 by Pool tensor_copy, 2/8 by ACT sign,
       2/8 by DVE copy. Output DMA triggers move from Pool(SWDGE) to the
       ACT HWDGE ring, freeing Pool for the copies.
    3. Tensor: single-tap matmul streams N=448 (3-level AP) instead of 512;
       DR matmuls stay 512 (they are LDWEIGHTS-bound anyway).
    """
    FP8 = mybir.dt.float8e4
    pitch = RP
    nrows = HP + 1  # 115: one dummy row absorbs the DR 2-element overrun
    stage_rows = N_STAGEROWS

    nc = bacc.Bacc(
        "TRN2", target_bir_lowering=False, debug=False, num_devices=N_CORES
    )
    x = nc.declare_dram_parameter("x", [BL, C, H, W], F32, isOutput=False)
    w = nc.declare_dram_parameter("weight", [C, C, 3, 3], F32, isOutput=False)
    y = nc.declare_dram_parameter("y", [BL, C, H, W], BF16, isOutput=True)

    with tile.TileContext(nc) as tc:
        with (
            tc.tile_pool(name="consts", bufs=1) as consts,
            tc.tile_pool(name="psum", bufs=1, space="PSUM") as psum_pool,
            tc.tile_pool(name="raw", bufs=1) as raw_pool,
            tc.tile_pool(name="xpad", bufs=1) as xpad_pool,
            tc.tile_pool(name="stage", bufs=3) as stage_pool,
        ):
            # ---- image-0 input loads issued before weight prep so the input
            # stream (the long pole at startup) begins immediately. First two
            # loads are 14 rows so the first Sign can start sooner.
            load_plan0 = [14, 14, 28, 28, 28]
            raws0 = []
            r0 = 0
            for rows in load_plan0:
                raw = raw_pool.tile([C, N_LOADROWS, W], F32, tag="raw", bufs=4)
                nc.sync.dma_start(raw[:, :rows, :], x[0][:, r0 : r0 + rows, :])
                raws0.append((r0, rows, raw))
                r0 += rows

            # ---- weight prep: scale[o], DR tap-pair tiles, kh=2 tiles ----
            wdr = consts.tile([C, 3, 2, C], FP8)  # pairs (0,kw)+(1,kw)
            wp2 = consts.tile([C, 2, C], FP8)  # pair (2,0)+(2,1) via planes
            w22 = consts.tile([C, C], FP8)  # tap (2,2)
            scale = consts.tile([C, 1], F32)
            identity = consts.tile([C, C], BF16)
            make_identity(nc, identity)
            with tc.tile_pool(name="wprep", bufs=1) as wp:
                wf = wp.tile([C, C, 3, 3], F32)
                nc.sync.dma_start(wf[:, :, :, :], w[:, :, :, :])
                wabs = wp.tile([C, C, 3, 3], F32)
                ssum = wp.tile([C, 1], F32)
                nc.scalar.activation(
                    wabs[:, :, :, :],
                    wf[:, :, :, :],
                    mybir.ActivationFunctionType.Abs,
                    accum_out=ssum[:, :],
                )
                nc.scalar.mul(scale[:, :], ssum[:, :], 1.0 / (C * 9))
                wsign = wp.tile([C, C, 3, 3], BF16)
                nc.scalar.sign(wsign[:, :, :, :], wf[:, :, :, :])
                for t, (kh, kw) in enumerate(TAPS):
                    pst = psum_pool.tile([C, C], BF16, tag="pst", bufs=2)
                    nc.tensor.transpose(pst[:, :], wsign[:, :, kh, kw], identity[:, :])
                    if kh < 2:
                        dst = wdr[:, kw, kh, :]
                    elif kw < 2:
                        dst = wp2[:, kw, :]
                    else:
                        dst = w22[:, :]
                    nc.vector.tensor_copy(dst, pst[:, :])

            # ---- persistent padded sign planes, double-buffered over images.
            # P0[r, 1+c] = sign(x[r-1, c]); P1[r, c] = P0[r, c+1]. Borders and
            # garbage cells zeroed once (interiors rewritten per image).
            xpads = []
            for k in range(2):
                xp = xpad_pool.tile(
                    [C, 2, nrows, pitch], FP8, tag=f"xpad{k}", name=f"xpad{k}"
                )
                xpads.append(xp)
                nc.gpsimd.memset(xp[:, 0, 0, :], 0.0)
                nc.gpsimd.memset(xp[:, 0, HP - 1 :, :], 0.0)
                nc.gpsimd.memset(xp[:, 0, :, W + 1 : pitch], 0.0)
                nc.gpsimd.memset(xp[:, 0, :, 0], 0.0)
                nc.gpsimd.memset(xp[:, 1, 0:2, :], 0.0)
                nc.gpsimd.memset(xp[:, 1, HP - 1 :, :], 0.0)
                nc.gpsimd.memset(xp[:, 1, :, W:pitch], 0.0)

            # P1-piece engine assignment by 14-row piece index (8 per image):
            # Pool copies most of it; ACT signs two pieces straight from raw;
            # DVE (busy with evacuation) takes two.
            P1_ENG = ["pool", "act", "pool", "dve", "pool", "act", "pool", "dve"]

            for n in range(BL):
                xim = x[n]
                yim = y[n]
                xpad = xpads[n % 2]
                if n == 0:
                    loads = raws0
                else:
                    loads = []
                    r0 = 0
                    for rows in [N_LOADROWS] * (H // N_LOADROWS):
                        raw = raw_pool.tile(
                            [C, N_LOADROWS, W], F32, tag="raw", bufs=4
                        )
                        nc.sync.dma_start(
                            raw[:, :rows, :], xim[:, r0 : r0 + rows, :]
                        )
                        loads.append((r0, rows, raw))
                        r0 += rows
                for r0, rows, raw in loads:
                    for a in range(0, rows, N_SIGNROWS):
                        rr = r0 + a + 1
                        piece = (r0 + a) // N_SIGNROWS
                        nc.scalar.sign(
                            xpad[:, 0, rr : rr + N_SIGNROWS, 1 : 1 + W],
                            raw[:, a : a + N_SIGNROWS, :],
                        )
                        eng = P1_ENG[piece]
                        if eng == "act":
                            nc.scalar.sign(
                                xpad[:, 1, rr : rr + N_SIGNROWS, 0:W],
                                raw[:, a : a + N_SIGNROWS, :],
                            )
                        else:
                            src = xpad[:, 0, rr : rr + N_SIGNROWS, 1 : 1 + W]
                            dst = xpad[:, 1, rr : rr + N_SIGNROWS, 0:W]
                            if eng == "pool":
                                nc.gpsimd.tensor_copy(dst, src)
                            else:
                                nc.vector.tensor_copy(dst, src)

                for s0 in range(0, H, stage_rows):
                    stage = stage_pool.tile([C, stage_rows, W], BF16, tag="stage")
                    for j in range(0, stage_rows, N_ROWCHUNK):
                        h0 = s0 + j
                        NF = N_ROWCHUNK * pitch
                        ps = psum_pool.tile([C, NF], F32, tag="ps", bufs=6)
                        for kw in range(3):
                            # taps (0,kw)+(1,kw) fused: K=256 DoubleRow
                            base = xpad[:, 0, h0, kw]
                            rhs = bass.AP(
                                tensor=base.tensor,
                                offset=base.offset,
                                ap=[base.ap[0], [pitch, 2], [1, NF]],
                            )
                            nc.tensor.matmul(
                                ps[:, :],
                                wdr[:, kw, :, :],
                                rhs,
                                start=(kw == 0),
                                stop=False,
                                perf_mode=mybir.MatmulPerfMode.DoubleRow,
                            )
                        # taps (2,0)+(2,1) fused across planes
                        base = xpad[:, 0, h0 + 2, 0]
                        rhs = bass.AP(
                            tensor=base.tensor,
                            offset=base.offset,
                            ap=[base.ap[0], [nrows * pitch, 2], [1, NF]],
                        )
                        nc.tensor.matmul(
                            ps[:, :],
                            wp2[:, :, :],
                            rhs,
                            start=False,
                            stop=False,
                            perf_mode=mybir.MatmulPerfMode.DoubleRow,
                        )
                        # tap (2,2): stream only the 448 useful columns
                        ps_rows = ps.rearrange("p (a b) -> p a b", b=pitch)[
                            :, :, 0:W
                        ]
                        base = xpad[:, 0, h0 + 2, 2]
                        rhs = bass.AP(
                            tensor=base.tensor,
                            offset=base.offset,
                            ap=[base.ap[0], [pitch, N_ROWCHUNK], [1, W]],
                        )
                        nc.tensor.matmul(
                            ps_rows, w22[:, :], rhs, start=False, stop=True
                        )
                        nc.vector.tensor_scalar_mul(
                            stage[:, j : j + N_ROWCHUNK, :], ps_rows, scale[:, :]
                        )
                    if n == BL - 1 and s0 == H - stage_rows:
                        # split the last store so the tail drains half as long
                        hs = stage_rows // 2
                        nc.scalar.dma_start(
                            yim[:, s0 : s0 + hs, :], stage[:, :hs, :]
                        )
                        nc.scalar.dma_start(
                            yim[:, s0 + hs : s0 + stage_rows, :], stage[:, hs:, :]
                        )
                    else:
                        nc.scalar.dma_start(
                            yim[:, s0 : s0 + stage_rows, :], stage[:, :, :]
                        )

    nc.compile()
    return nc


def build_nc(variant=None):
    variant = variant or VARIANT
    fp8 = variant in ("fp8dr", "fp8dr5", "fp8dr6", "fp8dr7", "fp8dr8")
    # fp8dr5: a second, column-shifted plane P1[r,c] = P0[r,c+1] lets taps
    # (2,0)+(2,1) share one DoubleRow matmul (pair step = plane stride), so a
    # chunk needs 5 matmuls instead of 6.
    planes = variant in ("fp8dr5", "fp8dr6", "fp8dr7", "fp8dr8")
    # fp8dr6: additionally (1) leave garbage-only pad cells (whose products
    # only ever land in discarded PSUM columns) unwritten, so the first
    # matmuls don't wait on slow strided memsets; (2) alternate the P1 fill
    # between ACT Sign and a DVE shift-copy to balance engine load; (3) store
    # output in 14-row pieces to shorten the kernel tail.
    lean = variant == "fp8dr6"
    stage_rows = 16 if lean else N_STAGEROWS
    # fp8dr7: fp8dr5 scheduling, but (1) buffer-1 border memsets deferred past
    # image 0 so buffer-0 init isn't queued behind them, (2) 56-row input
    # loads for images 1..3 (better DMA efficiency; image 0 keeps 28-row loads
    # for fast pipeline fill), (3) final store split to shorten the tail.
    lean7 = variant == "fp8dr7"
    # fp8dr8: ONLY the memset deferral from fp8dr7 (loads stay 28-row)
    defer = variant in ("fp8dr7", "fp8dr8")
    FP8 = mybir.dt.float8e4
    act_dt = FP8 if fp8 else BF16
    pitch = RP if fp8 else HP

    nc = bacc.Bacc(
        "TRN2", target_bir_lowering=False, debug=False, num_devices=N_CORES
    )
    x = nc.declare_dram_parameter("x", [BL, C, H, W], F32, isOutput=False)
    w = nc.declare_dram_parameter("weight", [C, C, 3, 3], F32, isOutput=False)
    y = nc.declare_dram_parameter("y", [BL, C, H, W], F32, isOutput=True)

    with tile.TileContext(nc) as tc:
        with (
            tc.tile_pool(name="consts", bufs=1) as consts,
            tc.tile_pool(name="psum", bufs=1, space="PSUM") as psum_pool,
        ):
            # ---- weight prep: scale[o] and transposed sign-weight tiles ----
            # bf16:  lhsT[i, tap, o] for the 9 taps
            # fp8dr: wdr[i, kw, j, o] pairs taps (kh=0,kw),(kh=1,kw); w2[i, kw, o]
            #        holds the kh=2 row
            if fp8:
                wdr = consts.tile([C, 3, 2, C], FP8)
                if planes:
                    wp2 = consts.tile([C, 2, C], FP8)  # taps (2,0),(2,1)
                    w22 = consts.tile([C, C], FP8)  # tap (2,2)
                else:
                    w2 = consts.tile([C, 3, C], FP8)
            else:
                lhsT = consts.tile([C, 9, C], BF16)  # [i, tap, o]
            scale = consts.tile([C, 1], F32)
            identity = consts.tile([C, C], BF16)
            make_identity(nc, identity)
            with tc.tile_pool(name="wprep", bufs=1) as wp:
                wf = wp.tile([C, C, 3, 3], F32)
                nc.sync.dma_start(wf[:, :, :, :], w[:, :, :, :])
                wabs = wp.tile([C, C, 3, 3], F32)
                ssum = wp.tile([C, 1], F32)
                nc.scalar.activation(
                    wabs[:, :, :, :],
                    wf[:, :, :, :],
                    mybir.ActivationFunctionType.Abs,
                    accum_out=ssum[:, :],
                )
                nc.scalar.mul(scale[:, :], ssum[:, :], 1.0 / (C * 9))
                wsign = wp.tile([C, C, 3, 3], BF16)
                nc.scalar.sign(wsign[:, :, :, :], wf[:, :, :, :])
                for t, (kh, kw) in enumerate(TAPS):
                    pst = psum_pool.tile([C, C], BF16, tag="pst", bufs=2)
                    nc.tensor.transpose(pst[:, :], wsign[:, :, kh, kw], identity[:, :])
                    if fp8 and planes:
                        if kh < 2:
                            dst = wdr[:, kw, kh, :]
                        elif kw < 2:
                            dst = wp2[:, kw, :]
                        else:
                            dst = w22[:, :]
                    elif fp8:
                        dst = wdr[:, kw, kh, :] if kh < 2 else w2[:, kw, :]
                    else:
                        dst = lhsT[:, t, :]
                    # DVE, not ACT: keeps ACT free for the first image's Sign
                    nc.vector.tensor_copy(dst, pst[:, :])

            # ---- main loop over local images ----
            with (
                tc.tile_pool(name="raw", bufs=2) as raw_pool,
                tc.tile_pool(name="xpad", bufs=1) as xpad_pool,
                tc.tile_pool(name="stage", bufs=3) as stage_pool,
            ):
                # Two persistent padded buffers, manually double-buffered
                # across images. Borders are zeroed ONCE here (the interior is
                # rewritten per image, borders stay zero), so image-boundary
                # matmuls never wait on memsets queued behind output DMAs.
                # fp8dr reads whole pitch-128 rows (N=512 contiguous spans);
                # one extra dummy row absorbs the last chunk's 2-element
                # overrun, and every non-interior cell is zeroed.
                nrows = HP + 1 if fp8 else HP
                nplanes = 2 if planes else 1

                def border_memsets(xp):
                    nc.gpsimd.memset(xp[:, 0, 0, :], 0.0)
                    nc.gpsimd.memset(xp[:, 0, HP - 1 :, :], 0.0)
                    nc.gpsimd.memset(xp[:, 0, :, W + 1 : pitch], 0.0)
                    nc.gpsimd.memset(xp[:, 0, :, 0], 0.0)
                    nc.gpsimd.memset(xp[:, 1, 0:2, :], 0.0)
                    nc.gpsimd.memset(xp[:, 1, HP - 1 :, :], 0.0)
                    nc.gpsimd.memset(xp[:, 1, :, W:pitch], 0.0)

                xpads = []
                for k in range(2):
                    xp = xpad_pool.tile(
                        [C, nplanes, nrows, pitch],
                        act_dt,
                        tag=f"xpad{k}",
                        name=f"xpad{k}",
                    )
                    xpads.append(xp)
                    if defer:
                        if k == 0:
                            border_memsets(xp)
                        continue
                    nc.gpsimd.memset(xp[:, 0, 0, :], 0.0)
                    if lean:
                        # thin true-pad strips on gpsimd (fast), fat
                        # garbage-only strips on the (idle-at-start) DVE, so
                        # buffer init never gates the first matmuls
                        nc.gpsimd.memset(xp[:, 0, HP - 1 :, :], 0.0)
                        nc.gpsimd.memset(xp[:, 0, 1 : HP - 1, 0], 0.0)
                        nc.gpsimd.memset(xp[:, 0, 1 : HP - 1, W + 1], 0.0)
                        nc.gpsimd.memset(xp[:, 1, HP - 1 :, :], 0.0)
                        nc.vector.memset(xp[:, 0, 1 : HP - 1, W + 2 : pitch], 0.0)
                        nc.vector.memset(xp[:, 1, 2 : HP - 1, W : pitch], 0.0)
                    elif fp8:
                        nc.gpsimd.memset(xp[:, 0, HP - 1 :, :], 0.0)
                        nc.gpsimd.memset(xp[:, 0, :, W + 1 : pitch], 0.0)
                        nc.gpsimd.memset(xp[:, 0, :, 0], 0.0)
                        if planes:
                            nc.gpsimd.memset(xp[:, 1, 0:2, :], 0.0)
                            nc.gpsimd.memset(xp[:, 1, HP - 1 :, :], 0.0)
                            nc.gpsimd.memset(xp[:, 1, :, W:pitch], 0.0)
                    else:
                        nc.gpsimd.memset(xp[:, 0, HP - 1, :], 0.0)
                        nc.gpsimd.memset(xp[:, 0, :, HP - 1], 0.0)
                        nc.gpsimd.memset(xp[:, 0, :, 0], 0.0)
                for n in range(BL):
                    xim = x[n]  # [C, H, W]
                    yim = y[n]
                    xpad = xpads[n % 2]
                    if lean7 and n > 0:
                        load_sizes = [56, 56]
                    else:
                        load_sizes = [N_LOADROWS] * (H // N_LOADROWS)
                    raw_rows = 56 if lean7 else N_LOADROWS
                    r0 = 0
                    for rows in load_sizes:
                        raw = raw_pool.tile(
                            [C, raw_rows, W], F32, tag="raw",
                            bufs=2 if lean7 else 4,
                        )
                        nc.sync.dma_start(
                            raw[:, :rows, :], xim[:, r0 : r0 + rows, :]
                        )
                        for a in range(0, rows, N_SIGNROWS):
                            rr = r0 + a + 1
                            nc.scalar.sign(
                                xpad[:, 0, rr : rr + N_SIGNROWS, 1 : 1 + W],
                                raw[:, a : a + N_SIGNROWS, :],
                            )
                            if planes and lean and (a // N_SIGNROWS) % 2 == 1:
                                # balance engines: every other P1 piece is a
                                # DVE shift-copy of P0 instead of an ACT Sign
                                nc.vector.tensor_copy(
                                    xpad[:, 1, rr : rr + N_SIGNROWS, 0:W],
                                    xpad[:, 0, rr : rr + N_SIGNROWS, 1 : 1 + W],
                                )
                            elif planes:
                                nc.scalar.sign(
                                    xpad[:, 1, rr : rr + N_SIGNROWS, 0:W],
                                    raw[:, a : a + N_SIGNROWS, :],
                                )
                        r0 += rows
                    if defer and n == 0:
                        # buffer 1 isn't read until image 1: zero its borders
                        # only now, so buffer 0's init wasn't queued behind it
                        border_memsets(xpads[1])
                    for s0 in range(0, H, stage_rows):
                        stage = stage_pool.tile([C, stage_rows, W], F32, tag="stage")
                        for j in range(0, stage_rows, N_ROWCHUNK):
                            h0 = s0 + j
                            if fp8:
                                # full-pitch output rows: N = 4*128 = 512 fp32
                                # (one PSUM bank); cols >= 112 of each row are
                                # garbage and skipped at evacuation
                                NF = N_ROWCHUNK * pitch
                                ps = psum_pool.tile([C, NF], F32, tag="ps", bufs=6)
                                for kw in range(3):
                                    # taps (0,kw)+(1,kw) fused: K=256 DoubleRow
                                    base = xpad[:, 0, h0, kw]
                                    rhs = bass.AP(
                                        tensor=base.tensor,
                                        offset=base.offset,
                                        ap=[base.ap[0], [pitch, 2], [1, NF]],
                                    )
                                    nc.tensor.matmul(
                                        ps[:, :],
                                        wdr[:, kw, :, :],
                                        rhs,
                                        start=(kw == 0),
                                        stop=False,
                                        perf_mode=mybir.MatmulPerfMode.DoubleRow,
                                    )
                                if planes:
                                    # taps (2,0)+(2,1) fused across the P0/P1
                                    # planes (pair step = plane stride)
                                    base = xpad[:, 0, h0 + 2, 0]
                                    rhs = bass.AP(
                                        tensor=base.tensor,
                                        offset=base.offset,
                                        ap=[base.ap[0], [nrows * pitch, 2], [1, NF]],
                                    )
                                    nc.tensor.matmul(
                                        ps[:, :],
                                        wp2[:, :, :],
                                        rhs,
                                        start=False,
                                        stop=False,
                                        perf_mode=mybir.MatmulPerfMode.DoubleRow,
                                    )
                                    base = xpad[:, 0, h0 + 2, 2]
                                    rhs = bass.AP(
                                        tensor=base.tensor,
                                        offset=base.offset,
                                        ap=[base.ap[0], [1, NF]],
                                    )
                                    nc.tensor.matmul(
                                        ps[:, :],
                                        w22[:, :],
                                        rhs,
                                        start=False,
                                        stop=True,
                                    )
                                else:
                                    for kw in range(3):
                                        # tap (2,kw)
                                        base = xpad[:, 0, h0 + 2, kw]
                                        rhs = bass.AP(
                                            tensor=base.tensor,
                                            offset=base.offset,
                                            ap=[base.ap[0], [1, NF]],
                                        )
                                        nc.tensor.matmul(
                                            ps[:, :],
                                            w2[:, kw, :],
                                            rhs,
                                            start=False,
                                            stop=(kw == 2),
                                        )
                                ps_rows = ps.rearrange(
                                    "p (a b) -> p a b", b=pitch
                                )[:, :, 0:W]
                            else:
                                ps = psum_pool.tile(
                                    [C, N_ROWCHUNK, W], F32, tag="ps", bufs=6
                                )
                                for t, (kh, kw) in enumerate(TAPS):
                                    nc.tensor.matmul(
                                        ps[:, :, :],
                                        lhsT[:, t, :],
                                        xpad[
                                            :,
                                            0,
                                            h0 + kh : h0 + kh + N_ROWCHUNK,
                                            kw : kw + W,
                                        ],
                                        start=(t == 0),
                                        stop=(t == len(TAPS) - 1),
                                    )
                                ps_rows = ps[:, :, :]
                            nc.vector.tensor_scalar_mul(
                                stage[:, j : j + N_ROWCHUNK, :], ps_rows, scale[:, :]
                            )
                        if lean7 and n == BL - 1 and s0 == H - stage_rows:
                            # split the very last store so the kernel tail only
                            # waits on half the bytes
                            hs = stage_rows // 2
                            nc.gpsimd.dma_start(
                                yim[:, s0 : s0 + hs, :], stage[:, :hs, :]
                            )
                            nc.gpsimd.dma_start(
                                yim[:, s0 + hs : s0 + stage_rows, :],
                                stage[:, hs:, :],
                            )
                        else:
                            nc.gpsimd.dma_start(
                                yim[:, s0 : s0 + stage_rows, :], stage[:, :, :]
                            )

    nc.compile()
    return nc


_NC_CACHE = {}


def _get_nc(variant=None):
    variant = variant or VARIANT
    if variant not in _NC_CACHE:
        if variant == "v2":
            _NC_CACHE[variant] = build_nc_v2()
        elif variant == "v3":
            _NC_CACHE[variant] = build_nc_v3()
        elif variant in ("v4", "v5", "v6", "v7"):
            _NC_CACHE[variant] = build_nc_v4(variant)
        else:
            _NC_CACHE[variant] = build_nc(variant)
    return _NC_CACHE[variant]


def kernel(
    x: np.ndarray,
    weight: np.ndarray,
    _trace: bool = False,
    _variant: str | None = None,
    **_kw,
):
    assert x.shape == (B, C, H, W) and weight.shape == (C, C, 3, 3)
    nc = _get_nc(_variant)
    xs = np.ascontiguousarray(x, dtype=np.float32)
    wgt = np.ascontiguousarray(weight, dtype=np.float32)
    in_maps = [
        {"x": xs[i * BL : (i + 1) * BL], "weight": wgt} for i in range(N_CORES)
    ]
    res = run_bass_kernel_spmd(
        nc, in_maps, core_ids=list(range(N_CORES)), trace=_trace
    )
    out = np.concatenate(
        [np.asarray(res.results[i]["y"], dtype=np.float32) for i in range(N_CORES)],
        axis=0,
    )
    if _trace:
        kernel.last_results = res
    return out

